# revision 15
# baseline (speedup 1.0000x reference)
"""MemN2N-style memory network on 8 TRN2 NeuronCores.

Math (see torch/jax module this mirrors):
    u = hidden.squeeze(1)                                  # [B, D]
    for h in range(3):
        E_A = bagsum(C[h][story])                          # [B, M, D]
        logit = einsum('bmd,bd->bm', E_A, u)
        p = softmax(logit, axis=1)
        E_C = bagsum(C[h+1][story])
        u = u + einsum('bm,bmd->bd', p, E_C)
    return logit, u

Key structure: E_A of hop h+1 == E_C of hop h, so only 4 gather+bag-sum
passes are needed (one per table), all with identical indices. We pack the
4 tables' rows side by side (bf16) so one 1KB-row dma_gather fetches all
four tables for a token. Bag-sum (10 tokens -> 1 bag) runs on the PE via
block-diagonal indicator matmuls accumulating in PSUM. The 3-hop attention
tail is tiny (B_local=4 per core) and runs on DVE/ACT/PE.

Sharding: data-parallel over batch B=32 across 8 cores (4 each); tables
replicated (bf16-packed, 32MB).
"""

import os

import numpy as np
import ml_dtypes

# ---- problem constants (hardcoded per harness contract) ----
B, M, S, V, D = 32, 512, 10, 32000, 128
HOPS = 3
NT = HOPS + 1            # 4 embedding tables
ROW = NT * D             # 512 bf16 elements per packed table row (1KB)
NCORES = 8
BL = B // NCORES         # 4 batches per core
NTOK = BL * M * S        # 20480 gathered tokens per core
CHUNKS = 16              # one PSUM tile (128 bags) per chunk
TPC = NTOK // CHUNKS     # 1280 tokens per chunk
BLKS = TPC // 128        # 10 gather blocks (of 128 tokens) per chunk
MT = M // 128            # 4 m-tiles per batch

TRACE = os.environ.get("BASS_KERNEL_TRACE", "0") == "1"

LAST_EXEC_TIME_NS = None
LAST_RESULTS = None

_NC_CACHE = {}


def _build_nc():
    import concourse.tile as tile
    from concourse import bacc, mybir
    import concourse.bass as bass  # noqa: F401
    from concourse.masks import make_identity

    f32 = mybir.dt.float32
    bf16 = mybir.dt.bfloat16
    i16 = mybir.dt.int16

    nc = bacc.Bacc("TRN2", target_bir_lowering=False, debug=False)

    tab = nc.dram_tensor("tab", [V, ROW], bf16, kind="ExternalInput")
    idx = nc.dram_tensor("idx", [128, NTOK // 16], i16, kind="ExternalInput")
    ind = nc.dram_tensor("ind", [128, BLKS * 128], bf16, kind="ExternalInput")
    u0 = nc.dram_tensor("u0", [1, BL * D], f32, kind="ExternalInput")
    logits_out = nc.dram_tensor("logits", [BL, M], f32, kind="ExternalOutput")
    u_out = nc.dram_tensor("u", [1, BL * D], f32, kind="ExternalOutput")

    with tile.TileContext(nc) as tc:
        with (
            tc.tile_pool(name="const", bufs=1) as cp,
            tc.tile_pool(name="gat", bufs=3) as gp,
            tc.tile_pool(name="emb", bufs=1) as ep,
            tc.tile_pool(name="work", bufs=2) as wp,
            tc.tile_pool(name="bag_psum", bufs=2, space="PSUM") as pp,
            tc.tile_pool(name="attn_psum", bufs=1, space="PSUM") as pa,
        ):
            # ---- constants into SBUF ----
            idx_sb = cp.tile([128, NTOK // 16], i16)
            nc.sync.dma_start(idx_sb[:], idx[:])
            ind_sb = cp.tile([128, BLKS * 128], bf16)
            nc.sync.dma_start(ind_sb[:], ind[:])
            u_sb = cp.tile([1, BL * D], f32)
            nc.sync.dma_start(u_sb[:], u0[:])
            ident = cp.tile([128, 128], f32)
            make_identity(nc, ident[:])
            ones1 = cp.tile([1, 128], f32)
            nc.vector.memset(ones1[:], 1.0)

            # E tables: [m-part, chunk, (table, d)], f32 for logits math,
            # bf16 copy for PE o-matmul rhs.
            E32 = ep.tile([128, CHUNKS, ROW], f32)
            E16 = ep.tile([128, CHUNKS, ROW], bf16)

            # ---- gather + bag-sum ----
            for g in range(CHUNKS):
                gt = gp.tile([128, BLKS, ROW], bf16, tag="gt")
                nc.gpsimd.dma_gather(
                    gt[:],
                    tab[:],
                    idx_sb[:, g * (TPC // 16):(g + 1) * (TPC // 16)],
                    TPC,
                    TPC,
                    ROW,
                    elem_step=ROW,
                    single_packet=False,
                )
                pt = pp.tile([128, ROW], f32, space="PSUM", tag="pt")
                for j in range(BLKS):
                    nc.tensor.matmul(
                        out=pt[:],
                        lhsT=ind_sb[:, j * 128:(j + 1) * 128],
                        rhs=gt[:, j, :],
                        start=(j == 0),
                        stop=(j == BLKS - 1),
                    )
                nc.vector.tensor_copy(E32[:, g, :], pt[:])
                nc.scalar.mul(E16[:, g, :], pt[:], 1.0)

            # ---- 3-hop attention (tiny; B_local=4) ----
            lt_final = cp.tile([BL, M], f32)
            for h in range(HOPS):
                # replicate u row across 128 partitions via rank-1 matmul
                urep = pa.tile([128, BL * D], f32, space="PSUM", tag="urep")
                nc.tensor.matmul(
                    out=urep[:],
                    lhsT=ones1[:],
                    rhs=u_sb[:],
                    start=True,
                    stop=True,
                )
                # logits[m, (mt,b)] = sum_d E_A[m, d] * u[b, d]
                lcols = wp.tile([128, BL * MT], f32, tag="lcols")
                for b in range(BL):
                    for mt in range(MT):
                        gidx = b * MT + mt
                        scratch = wp.tile([128, D], f32, tag="scratch")
                        nc.vector.tensor_tensor(
                            out=scratch[:],
                            in0=E32[:, gidx, h * D:(h + 1) * D],
                            in1=urep[:, b * D:(b + 1) * D],
                            op=mybir.AluOpType.mult,
                        )
                        nc.vector.tensor_reduce(
                            out=lcols[:, MT * mt + b:MT * mt + b + 1],
                            in_=scratch[:],
                            axis=mybir.AxisListType.X,
                            op=mybir.AluOpType.add,
                        )
                # transpose to [b, m]
                ltp = pa.tile([BL, M], f32, space="PSUM", tag="ltp")
                for mt in range(MT):
                    nc.tensor.transpose(
                        out=ltp[:, mt * 128:(mt + 1) * 128],
                        in_=lcols[:, MT * mt:MT * (mt + 1)],
                        identity=ident[:],
                    )
                if h == HOPS - 1:
                    nc.vector.tensor_copy(lt_final[:], ltp[:])

                # softmax (unnormalized; normalization folded into o)
                mx = wp.tile([BL, 1], f32, tag="mx")
                nc.vector.tensor_reduce(
                    out=mx[:], in_=ltp[:], axis=mybir.AxisListType.X,
                    op=mybir.AluOpType.max, negate=True,
                )
                et = wp.tile([BL, M], f32, tag="et")
                zs = wp.tile([BL, 1], f32, tag="zs")
                nc.scalar.activation(
                    out=et[:], in_=ltp[:],
                    func=mybir.ActivationFunctionType.Exp,
                    bias=mx[:], scale=1.0, accum_out=zs[:],
                )
                rz = wp.tile([BL, 1], f32, tag="rz")
                nc.vector.reciprocal(rz[:], zs[:])
                # transpose rz [4,1] -> [1,4] so per-b scalars sit on partition 0
                rzt_p = pa.tile([1, BL], f32, space="PSUM", tag="rzt_p")
                nc.tensor.transpose(
                    out=rzt_p[:], in_=rz[:], identity=ident[:BL, :BL]
                )
                rzt = wp.tile([1, BL], f32, tag="rzt")
                nc.vector.tensor_copy(rzt[:], rzt_p[:])
                # transpose probs back to [m, (mt, b)]
                ptp = pa.tile([128, BL * MT], f32, space="PSUM", tag="ptp")
                for mt in range(MT):
                    nc.tensor.transpose(
                        out=ptp[:, MT * mt:MT * (mt + 1)],
                        in_=et[:, mt * 128:(mt + 1) * 128],
                        identity=ident[:BL, :BL],
                    )
                pt_sb = wp.tile([128, BL * MT], bf16, tag="pt_sb")
                nc.vector.tensor_copy(pt_sb[:], ptp[:])
                # o[b, d] = sum_m p[m, b] * E_C[m, d], packed as one [1, BL*D] row
                op_ = pa.tile([1, BL * D], f32, space="PSUM", tag="op")
                for b in range(BL):
                    for mt in range(MT):
                        nc.tensor.matmul(
                            out=op_[:, b * D:(b + 1) * D],
                            lhsT=pt_sb[:, MT * mt + b:MT * mt + b + 1],
                            rhs=E16[:, b * MT + mt, (h + 1) * D:(h + 2) * D],
                            start=(mt == 0),
                            stop=(mt == MT - 1),
                        )
                # u = u + o / Z  (per-b normalization scalar from rzt)
                osb = wp.tile([1, BL * D], f32, tag="osb")
                for b in range(BL):
                    nc.vector.tensor_scalar_mul(
                        osb[:, b * D:(b + 1) * D],
                        op_[:, b * D:(b + 1) * D],
                        rzt[:, b:b + 1],
                    )
                u_new = cp.tile([1, BL * D], f32, tag=f"u{h}")
                nc.vector.tensor_add(u_new[:], u_sb[:], osb[:])
                u_sb = u_new

            nc.sync.dma_start(logits_out[:], lt_final[:])
            nc.sync.dma_start(u_out[:], u_sb[:])

    nc.compile()
    return nc


def _get_nc():
    if "nc" not in _NC_CACHE:
        _NC_CACHE["nc"] = _build_nc()
    return _NC_CACHE["nc"]


def _indicator_host():
    # ind[p, j*128 + k] = 1 iff token (j*128+p) of a 1280-token chunk
    # belongs to bag k (bags are 10 consecutive tokens)
    p = np.arange(128)[:, None]
    out = np.zeros((128, BLKS * 128), dtype=ml_dtypes.bfloat16)
    for j in range(BLKS):
        bag = (128 * j + p[:, 0]) // S
        out[np.arange(128), j * 128 + bag] = 1.0
    return out


def kernel(story, hidden, C):
    global LAST_EXEC_TIME_NS, LAST_RESULTS
    story = np.asarray(story)
    hidden = np.asarray(hidden, dtype=np.float32)
    C = np.asarray(C, dtype=np.float32)
    assert story.shape == (B, M, S) and C.shape == (NT, V, D)

    nc = _get_nc()
    from concourse.bass_utils import run_bass_kernel_spmd

    # pack the 4 tables' rows side by side, bf16
    tab = np.ascontiguousarray(C.transpose(1, 0, 2).reshape(V, ROW)).astype(
        ml_dtypes.bfloat16
    )
    ind = _indicator_host()

    in_maps = []
    for c in range(NCORES):
        toks = story[c * BL:(c + 1) * BL].reshape(-1).astype(np.int16)
        idxs = np.ascontiguousarray(np.tile(toks.reshape(-1, 16).T, (8, 1)))
        u0c = np.ascontiguousarray(
            hidden[c * BL:(c + 1) * BL, 0, :].reshape(1, BL * D)
        )
        in_maps.append({"tab": tab, "idx": idxs, "ind": ind, "u0": u0c})

    res = run_bass_kernel_spmd(nc, in_maps, list(range(NCORES)), trace=TRACE)
    LAST_RESULTS = res
    LAST_EXEC_TIME_NS = res.exec_time_ns

    logits = np.concatenate(
        [np.asarray(res.results[c]["logits"]) for c in range(NCORES)], axis=0
    ).astype(np.float32)
    u = np.concatenate(
        [np.asarray(res.results[c]["u"]).reshape(BL, D) for c in range(NCORES)],
        axis=0,
    ).astype(np.float32)
    return logits, u


# revision 16
# speedup vs baseline: 1.7962x; 1.7962x over previous
"""MemN2N-style memory network on 8 TRN2 NeuronCores.

Math (see torch/jax module this mirrors):
    u = hidden.squeeze(1)                                  # [B, D]
    for h in range(3):
        E_A = bagsum(C[h][story])                          # [B, M, D]
        logit = einsum('bmd,bd->bm', E_A, u)
        p = softmax(logit, axis=1)
        E_C = bagsum(C[h+1][story])
        u = u + einsum('bm,bmd->bd', p, E_C)
    return logit, u

Key structure: E_A of hop h+1 == E_C of hop h, so only 4 gather+bag-sum
passes are needed (one per table), all with identical indices. We pack the
4 tables' rows side by side (bf16) so one 1KB-row dma_gather fetches all
four tables for a token. Bag-sum (10 tokens -> 1 bag) runs on the PE via
block-diagonal indicator matmuls accumulating in PSUM. The 3-hop attention
tail is tiny (B_local=4 per core) and runs on DVE/ACT/PE.

Perf notes (measured on HW):
  - SWDGE descriptor generation is the bottleneck (~8ns/descriptor on one
    Q7 core-pair); with num_swdge_queues=4 and gathers striped across the
    4 queues, 4 core-pairs generate concurrently.
  - single_packet gathers crash above ~512 indices; 512-idx chunks with
    single_packet=True across 4 queues measured fastest (~95us for all
    gathers vs 200us for 16x1280 on one queue).
  - deep gather pool (bufs) needed so WAR tile reuse doesn't serialize.

Sharding: data-parallel over batch B=32 across 8 cores (4 each); tables
replicated (bf16-packed, 32MB).
"""

import os

import numpy as np
import ml_dtypes

# ---- problem constants (hardcoded per harness contract) ----
B, M, S, V, D = 32, 512, 10, 32000, 128
HOPS = 3
NT = HOPS + 1            # 4 embedding tables
ROW = NT * D             # 512 bf16 elements per packed table row (1KB)
NCORES = 8
BL = B // NCORES         # 4 batches per core
NTOK = BL * M * S        # 20480 gathered tokens per core
GCHUNK = 512             # tokens per dma_gather (single_packet-safe)
NG = NTOK // GCHUNK      # 40 gather instructions
GBLK = GCHUNK // 128     # 4 blocks of 128 tokens per gather tile
CHUNKS = 16              # one PSUM tile (128 bags) per bag-sum group
TPC = NTOK // CHUNKS     # 1280 tokens per group
BLKS = TPC // 128        # 10 blocks of 128 tokens per group
MT = M // 128            # 4 m-tiles per batch
NQ = 4                   # SWDGE queues

TRACE = os.environ.get("BASS_KERNEL_TRACE", "0") == "1"

LAST_EXEC_TIME_NS = None
LAST_RESULTS = None

_NC_CACHE = {}


def _build_nc():
    import concourse.tile as tile
    from concourse import bacc, mybir
    import concourse.bass as bass
    from concourse.masks import make_identity

    f32 = mybir.dt.float32
    bf16 = mybir.dt.bfloat16
    i16 = mybir.dt.int16

    nc = bacc.Bacc("TRN2", target_bir_lowering=False, debug=False,
                   num_swdge_queues=NQ)

    tab = nc.dram_tensor("tab", [V, ROW], bf16, kind="ExternalInput")
    idx = nc.dram_tensor("idx", [128, NTOK // 16], i16, kind="ExternalInput")
    ind = nc.dram_tensor("ind", [128, BLKS * 128], bf16, kind="ExternalInput")
    u0 = nc.dram_tensor("u0", [1, BL * D], f32, kind="ExternalInput")
    logits_out = nc.dram_tensor("logits", [BL, M], f32, kind="ExternalOutput")
    u_out = nc.dram_tensor("u", [1, BL * D], f32, kind="ExternalOutput")

    def bcast_mid(ap, n):
        # [P, F] -> [P, n, F] with a stride-0 middle dim
        return bass.AP(ap.tensor, ap.offset, [ap.ap[0], [0, n], ap.ap[1]])

    def stride_cols(ap_full, start, step, n):
        # [P, F] -> [P, n] taking columns start, start+step, ...
        return bass.AP(
            ap_full.tensor, ap_full.offset + start, [ap_full.ap[0], [step, n]]
        )

    with tile.TileContext(nc) as tc:
        with (
            tc.tile_pool(name="const", bufs=1) as cp,
            tc.tile_pool(name="gat", bufs=24) as gp,
            tc.tile_pool(name="emb", bufs=1) as ep,
            tc.tile_pool(name="work", bufs=2) as wp,
            tc.tile_pool(name="bag_psum", bufs=2, space="PSUM") as pp,
            tc.tile_pool(name="attn_psum", bufs=1, space="PSUM") as pa,
        ):
            # idx first: gathers depend only on it
            idx_sb = cp.tile([128, NTOK // 16], i16)
            nc.sync.dma_start(idx_sb[:], idx[:])

            # ---- gathers: 40 x 512 idxs, striped over 4 SWDGE queues ----
            gts = []
            for i in range(NG):
                gt = gp.tile([128, GBLK, ROW], bf16, tag="gt")
                nc.gpsimd.dma_gather(
                    gt[:],
                    tab[:],
                    idx_sb[:, i * (GCHUNK // 16):(i + 1) * (GCHUNK // 16)],
                    GCHUNK,
                    GCHUNK,
                    ROW,
                    elem_step=ROW,
                    single_packet=True,
                    queue_num=i % NQ,
                )
                gts.append(gt)

            # ---- other constants (load while gathers run) ----
            ind_sb = cp.tile([128, BLKS * 128], bf16)
            nc.sync.dma_start(ind_sb[:], ind[:])
            u_sb = cp.tile([1, BL * D], f32)
            nc.sync.dma_start(u_sb[:], u0[:])
            ident = cp.tile([128, 128], f32)
            make_identity(nc, ident[:])
            ones1 = cp.tile([1, 128], f32)
            nc.vector.memset(ones1[:], 1.0)

            # E tables: [m-part, group, (table, d)]
            E32 = ep.tile([128, CHUNKS, ROW], f32)
            E16 = ep.tile([128, CHUNKS, ROW], bf16)

            # ---- bag-sum: group g = bags [128g, 128g+128) = blocks 10g..10g+9
            for g in range(CHUNKS):
                pt = pp.tile([128, ROW], f32, space="PSUM", tag="pt")
                for j in range(BLKS):
                    k = BLKS * g + j          # global block index
                    nc.tensor.matmul(
                        out=pt[:],
                        lhsT=ind_sb[:, j * 128:(j + 1) * 128],
                        rhs=gts[k // GBLK][:, k % GBLK, :],
                        start=(j == 0),
                        stop=(j == BLKS - 1),
                    )
                nc.vector.tensor_copy(E32[:, g, :], pt[:])
                nc.scalar.mul(E16[:, g, :], pt[:], 1.0)

            # ---- 3-hop attention (tiny; B_local=4) ----
            lt_final = cp.tile([BL, M], f32)
            for h in range(HOPS):
                # replicate u row across 128 partitions via rank-1 matmul
                urep = pa.tile([128, BL * D], f32, space="PSUM", tag="urep")
                nc.tensor.matmul(
                    out=urep[:], lhsT=ones1[:], rhs=u_sb[:],
                    start=True, stop=True,
                )
                # logits[m, 4b+mt] = sum_d E_A[m, (b,mt), d] * u[b, d]
                lcols = wp.tile([128, BL * MT], f32, tag="lcols")
                for b in range(BL):
                    scratch = wp.tile([128, MT, D], f32, tag="scratch")
                    nc.vector.tensor_tensor(
                        out=scratch[:],
                        in0=E32[:, b * MT:(b + 1) * MT, h * D:(h + 1) * D],
                        in1=bcast_mid(urep[:, b * D:(b + 1) * D], MT),
                        op=mybir.AluOpType.mult,
                    )
                    nc.vector.tensor_reduce(
                        out=lcols[:, MT * b:MT * (b + 1)],
                        in_=scratch[:],
                        axis=mybir.AxisListType.X,
                        op=mybir.AluOpType.add,
                    )
                # transpose to [b, m]; lcols is b-major so mt slice is strided
                ltp = pa.tile([BL, M], f32, space="PSUM", tag="ltp")
                for mt in range(MT):
                    nc.tensor.transpose(
                        out=ltp[:, mt * 128:(mt + 1) * 128],
                        in_=stride_cols(lcols[:], mt, MT, BL),
                        identity=ident[:],
                    )
                if h == HOPS - 1:
                    nc.vector.tensor_copy(lt_final[:], ltp[:])

                # softmax (unnormalized; normalization folded into o)
                mx = wp.tile([BL, 1], f32, tag="mx")
                nc.vector.tensor_reduce(
                    out=mx[:], in_=ltp[:], axis=mybir.AxisListType.X,
                    op=mybir.AluOpType.max, negate=True,
                )
                et = wp.tile([BL, M], f32, tag="et")
                zs = wp.tile([BL, 1], f32, tag="zs")
                nc.scalar.activation(
                    out=et[:], in_=ltp[:],
                    func=mybir.ActivationFunctionType.Exp,
                    bias=mx[:], scale=1.0, accum_out=zs[:],
                )
                rz = wp.tile([BL, 1], f32, tag="rz")
                nc.vector.reciprocal(rz[:], zs[:])
                # rz [4,1] -> [1,4] so per-b scalars sit on partition 0
                rzt_p = pa.tile([1, BL], f32, space="PSUM", tag="rzt_p")
                nc.tensor.transpose(
                    out=rzt_p[:], in_=rz[:], identity=ident[:BL, :BL]
                )
                rzt = wp.tile([1, BL], f32, tag="rzt")
                nc.vector.tensor_copy(rzt[:], rzt_p[:])
                # transpose probs back to [m, (mt, b)]
                ptp = pa.tile([128, BL * MT], f32, space="PSUM", tag="ptp")
                for mt in range(MT):
                    nc.tensor.transpose(
                        out=ptp[:, MT * mt:MT * (mt + 1)],
                        in_=et[:, mt * 128:(mt + 1) * 128],
                        identity=ident[:BL, :BL],
                    )
                pt_sb = wp.tile([128, BL * MT], bf16, tag="pt_sb")
                nc.vector.tensor_copy(pt_sb[:], ptp[:])
                # o[b, d] = sum_m p[m, b] * E_C[m, d], packed as one [1, BL*D] row
                op_ = pa.tile([1, BL * D], f32, space="PSUM", tag="op")
                for b in range(BL):
                    for mt in range(MT):
                        nc.tensor.matmul(
                            out=op_[:, b * D:(b + 1) * D],
                            lhsT=pt_sb[:, MT * mt + b:MT * mt + b + 1],
                            rhs=E16[:, b * MT + mt, (h + 1) * D:(h + 2) * D],
                            start=(mt == 0),
                            stop=(mt == MT - 1),
                        )
                # u = u + o / Z  (per-b normalization scalar from rzt)
                osb = wp.tile([1, BL * D], f32, tag="osb")
                for b in range(BL):
                    nc.vector.tensor_scalar_mul(
                        osb[:, b * D:(b + 1) * D],
                        op_[:, b * D:(b + 1) * D],
                        rzt[:, b:b + 1],
                    )
                u_new = cp.tile([1, BL * D], f32, tag=f"u{h}")
                nc.vector.tensor_add(u_new[:], u_sb[:], osb[:])
                u_sb = u_new

            nc.sync.dma_start(logits_out[:], lt_final[:])
            nc.sync.dma_start(u_out[:], u_sb[:])

    nc.compile()
    return nc


def _get_nc():
    if "nc" not in _NC_CACHE:
        _NC_CACHE["nc"] = _build_nc()
    return _NC_CACHE["nc"]


def _indicator_host():
    # ind[p, j*128 + k] = 1 iff token (j*128+p) of a 1280-token group
    # belongs to bag k (bags are 10 consecutive tokens)
    p = np.arange(128)
    out = np.zeros((128, BLKS * 128), dtype=ml_dtypes.bfloat16)
    for j in range(BLKS):
        bag = (128 * j + p) // S
        out[p, j * 128 + bag] = 1.0
    return out


def kernel(story, hidden, C):
    global LAST_EXEC_TIME_NS, LAST_RESULTS
    story = np.asarray(story)
    hidden = np.asarray(hidden, dtype=np.float32)
    C = np.asarray(C, dtype=np.float32)
    assert story.shape == (B, M, S) and C.shape == (NT, V, D)

    nc = _get_nc()
    from concourse.bass_utils import run_bass_kernel_spmd

    # pack the 4 tables' rows side by side, bf16
    tab = np.ascontiguousarray(C.transpose(1, 0, 2).reshape(V, ROW)).astype(
        ml_dtypes.bfloat16
    )
    ind = _indicator_host()

    in_maps = []
    for c in range(NCORES):
        toks = story[c * BL:(c + 1) * BL].reshape(-1).astype(np.int16)
        idxs = np.ascontiguousarray(np.tile(toks.reshape(-1, 16).T, (8, 1)))
        u0c = np.ascontiguousarray(
            hidden[c * BL:(c + 1) * BL, 0, :].reshape(1, BL * D)
        )
        in_maps.append({"tab": tab, "idx": idxs, "ind": ind, "u0": u0c})

    res = run_bass_kernel_spmd(nc, in_maps, list(range(NCORES)), trace=TRACE)
    LAST_RESULTS = res
    LAST_EXEC_TIME_NS = res.exec_time_ns

    logits = np.concatenate(
        [np.asarray(res.results[c]["logits"]) for c in range(NCORES)], axis=0
    ).astype(np.float32)
    u = np.concatenate(
        [np.asarray(res.results[c]["u"]).reshape(BL, D) for c in range(NCORES)],
        axis=0,
    ).astype(np.float32)
    return logits, u
